# revision 6
# baseline (speedup 1.0000x reference)
"""BFPLinear Trainium2 kernel.

Computes: out = bfp_quantize(x) @ bfp_quantize(w).T + 2*bias
where bfp_quantize is 8-bit block-floating-point with shared-exponent
groups of 32 along the last (in_features) dim.

Sharding across 8 NeuronCores: 4 batch-groups x 2 column-groups.
Each core gets x[2048, 4096], w[2048, 4096], bias2[2048] and produces
out[2048, 2048].

Quantization is 3 passes over the data (vs 5 in the naive scheme):
  1. DVE/GPSIMD grouped abs-max reduce (groups of 32 along free dim)
  2. t = x + C  where C = 1.5*2^23 * step encodes round-to-step:
     C_bits = (gmax_bits & 0x7F800000) + 0x08C00000
     (exponent of gmax, then exponent += 17 and mantissa 1.5)
     fp32 RNE at ulp(t) = step rounds x to the step grid.
  3. xq_bf16 = t - C  (exact; |q| <= 128 is bf16-exact)
The reference clips q to +-127; values that round to +-128 (elements
within 0.4% of their group max) differ by one step -- contributes
~1e-3 relative error at the output scale, well under tolerance.

Per 128-row strip: xbar DMA transpose -> [K partitions, rows] bf16
tiles, PE matmul accumulate over K into PSUM, bias add on evict.
"""

import os
import numpy as np

import concourse.bass as bass
import concourse.bacc as bacc
import concourse.tile as tile
import concourse.mybir as mybir
from concourse.bass_utils import run_bass_kernel_spmd

F32 = mybir.dt.float32
BF16 = mybir.dt.bfloat16
U32 = mybir.dt.uint32
ALU = mybir.AluOpType
AX = mybir.AxisListType

# Full problem
B_FULL, IN_FULL, OUT_FULL = 8192, 4096, 4096
NBATCH, NCOL = 4, 2  # 4 batch-groups x 2 col-groups = 8 cores
SM_FULL = B_FULL // NBATCH    # 2048 rows of x per core
SN_FULL = OUT_FULL // NCOL    # 2048 output cols per core

GROUP = 32
# C_bits = gexp_bits + (17 << 23) + 0x00400000 (mantissa 1.5)
C_ADD = 0x08C00000


def _bcast_group(t_ap, g, e=GROUP):
    """View a [128, g] tile as [128, g, e] with the inner dim broadcast."""
    return bass.AP(
        tensor=t_ap.tensor,
        offset=t_ap.offset,
        ap=[t_ap.ap[0], t_ap.ap[1], [0, e]],
    )


def build_bass(SM=SM_FULL, SN=SN_FULL, K=IN_FULL, CH=1024):
    """Build the per-core Bass program.

    SM: rows of x shard; SN: rows of w shard (= output cols);
    K: contraction dim; CH: quantization chunk size (divides K,
    multiple of 128).
    """
    assert K % CH == 0 and CH % 128 == 0 and CH % GROUP == 0
    NKT = K // 128          # k-tiles
    CHT = CH // 128         # k-tiles per chunk
    G = CH // GROUP         # groups per chunk
    NCH = K // CH           # chunks per strip
    MS = SM // 128          # m-strips
    NS = SN // 128          # n-strips (w row strips)
    NSL = (SN + 511) // 512  # 512-wide n slices per psum

    nc = bacc.Bacc("TRN2", target_bir_lowering=False)

    x = nc.dram_tensor("x", [SM, K], F32, kind="ExternalInput")
    w = nc.dram_tensor("w", [SN, K], F32, kind="ExternalInput")
    b2 = nc.dram_tensor("b2", [SN], F32, kind="ExternalInput")
    o = nc.dram_tensor("o", [SM, SN], F32, kind="ExternalOutput")

    with tile.TileContext(nc) as tc:
        with (
            tc.tile_pool(name="res", bufs=1) as res_p,
            tc.tile_pool(name="nat", bufs=4) as nat_p,
            tc.tile_pool(name="tp", bufs=3) as t_p,
            tc.tile_pool(name="qb", bufs=4) as qb_p,
            tc.tile_pool(name="tiny", bufs=8) as tiny_p,
            tc.tile_pool(name="xqt", bufs=2) as xqt_p,
            tc.tile_pool(name="outp", bufs=2) as out_p,
            tc.tile_pool(name="psum", bufs=2, space="PSUM") as psum_p,
        ):
            # per-k-chunk resident quantized-transposed weights
            wqT = [res_p.tile([128, CHT, SN], BF16, tag=f"wq{h}",
                              name=f"wq{h}")
                   for h in range(NCH)]
            # bias row (bf16) + ones column for the PE bias-accumulate
            bias2b = res_p.tile([1, SN], BF16)
            ones = res_p.tile([1, 128], BF16)
            nc.gpsimd.dma_start(
                out=bias2b,
                in_=bass.AP(tensor=b2, offset=0, ap=[[0, 1], [1, SN]]),
            )
            nc.vector.memset(ones, 1.0)

            def quant_chunk(src_slice, dst3d):
                """Quantize one [128, CH] fp32 chunk and write its
                transposed bf16 k-tiles into dst3d ([128, CHT, 128])."""
                nat = nat_p.tile([128, CH], F32, tag="nat")
                nc.sync.dma_start(out=nat, in_=src_slice)
                nat3 = nat[:].rearrange("p (g e) -> p g e", e=GROUP)

                gmax = tiny_p.tile([128, G], F32, tag="gmax")
                nc.vector.tensor_reduce(
                    out=gmax[:], in_=nat3, axis=AX.X, op=ALU.max,
                    apply_absolute_value=True,
                )
                # C = 1.5 * 2^23 * step, built from the exponent bits
                # (walrus rejects bitwise+arith in one dual-op ts)
                nc.vector.tensor_scalar(
                    out=gmax[:].bitcast(U32), in0=gmax[:].bitcast(U32),
                    scalar1=0x7F800000, scalar2=None, op0=ALU.bitwise_and,
                )
                nc.vector.tensor_scalar(
                    out=gmax[:].bitcast(U32), in0=gmax[:].bitcast(U32),
                    scalar1=C_ADD, scalar2=None, op0=ALU.add,
                )
                cb = _bcast_group(gmax[:], G)

                t = t_p.tile([128, CH], F32, tag="t")
                t3 = t[:].rearrange("p (g e) -> p g e", e=GROUP)
                nc.gpsimd.tensor_tensor(out=t3, in0=nat3, in1=cb, op=ALU.add)

                qb = qb_p.tile([128, CH], BF16, tag="qb")
                qb3 = qb[:].rearrange("p (g e) -> p g e", e=GROUP)
                nc.vector.tensor_tensor(out=qb3, in0=t3, in1=cb,
                                        op=ALU.subtract)
                # blocked xbar transpose: [128, CH] -> [128, CHT, 128]
                nc.scalar.dma_start_transpose(out=dst3d, in_=qb[:])

            # ---- prefetch-quantize x strips 0,1 so PE can start early
            xqt_tiles = {}

            def quant_x_strip(m):
                xqt = xqt_p.tile([128, NKT, 128], BF16, tag="xqt")
                xqt_tiles[m] = xqt
                for h in range(NCH):
                    quant_chunk(
                        x[m * 128:(m + 1) * 128, h * CH:(h + 1) * CH],
                        xqt[:, h * CHT:(h + 1) * CHT, :],
                    )

            quant_x_strip(0)
            quant_x_strip(1)

            # ---- W phase: quantize+transpose w, k-chunk-major so the
            # matmul k-loop can start after the first chunk
            for h in range(NCH):
                for s in range(NS):
                    quant_chunk(
                        w[s * 128:(s + 1) * 128, h * CH:(h + 1) * CH],
                        wqT[h][:, :, s * 128:(s + 1) * 128],
                    )

            # ---- X phase: per m-strip quantize, transpose, matmul, evict
            for m in range(MS):
                if m not in xqt_tiles:
                    quant_x_strip(m)
                xqt = xqt_tiles[m]
                psum = psum_p.tile([128, SN], F32, tag="psum")
                for nj in range(NSL):
                    n0 = nj * 512
                    n1 = min(SN, n0 + 512)
                    # seed PSUM with the (doubled) bias via a K=1 matmul
                    nc.tensor.matmul(
                        psum[:, n0:n1],
                        ones[:],
                        bias2b[:, n0:n1],
                        start=True,
                        stop=False,
                    )
                for kt in range(NKT):
                    wt = wqT[kt // CHT]
                    kk = kt % CHT
                    for nj in range(NSL):
                        n0 = nj * 512
                        n1 = min(SN, n0 + 512)
                        nc.tensor.matmul(
                            psum[:, n0:n1],
                            xqt[:, kt, :],
                            wt[:, kk, n0:n1],
                            start=False,
                            stop=(kt == NKT - 1),
                        )
                outt = out_p.tile([128, SN], F32, tag="outt")
                nc.scalar.copy(out=outt[:], in_=psum[:])
                nc.sync.dma_start(
                    out=o[m * 128:(m + 1) * 128, :], in_=outt[:]
                )

    nc.compile()
    return nc


_NC_CACHE = {}


def _get_nc(key=("full",)):
    if key not in _NC_CACHE:
        if key == ("full",):
            _NC_CACHE[key] = build_bass()
        else:
            _NC_CACHE[key] = build_bass(*key)
    return _NC_CACHE[key]


def kernel(input, weight, bias):
    input = np.ascontiguousarray(input, dtype=np.float32)
    weight = np.ascontiguousarray(weight, dtype=np.float32)
    bias = np.ascontiguousarray(bias, dtype=np.float32)

    nc = _get_nc()
    b2_full = bias * np.float32(2.0)

    in_maps = []
    for c in range(8):
        bi, ni = divmod(c, NCOL)
        in_maps.append({
            "x": input[bi * SM_FULL:(bi + 1) * SM_FULL, :],
            "w": weight[ni * SN_FULL:(ni + 1) * SN_FULL, :],
            "b2": b2_full[ni * SN_FULL:(ni + 1) * SN_FULL],
        })

    trace = bool(int(os.environ.get("BFP_TRACE", "0")))
    res = run_bass_kernel_spmd(
        nc, in_maps, core_ids=list(range(8)), trace=trace,
    )
    kernel.last_results = res

    out = np.empty((B_FULL, OUT_FULL), dtype=np.float32)
    for c in range(8):
        bi, ni = divmod(c, NCOL)
        out[bi * SM_FULL:(bi + 1) * SM_FULL,
            ni * SN_FULL:(ni + 1) * SN_FULL] = res.results[c]["o"]
    return out


def _make_runner(nc):
    import jax
    from jax.sharding import Mesh, PartitionSpec
    from jax.experimental.shard_map import shard_map
    from concourse import bass2jax as b2j
    import concourse.mybir as mybir_

    b2j.install_neuronx_cc_hook()
    partition_name = (
        nc.partition_id_tensor.name if nc.partition_id_tensor else None
    )
    in_names, out_names, out_avals = [], [], []
    for alloc in nc.m.functions[0].allocations:
        if not isinstance(alloc, mybir_.MemoryLocationSet):
            continue
        name = alloc.memorylocations[0].name
        if alloc.kind == "ExternalInput":
            if name != partition_name:
                in_names.append(name)
        elif alloc.kind == "ExternalOutput":
            out_names.append(name)
            out_avals.append(jax.core.ShapedArray(
                tuple(alloc.tensor_shape), mybir_.dt.np(alloc.dtype)))
    n_params = len(in_names)
    all_names = list(in_names) + list(out_names)
    if partition_name is not None:
        all_names.append(partition_name)

    def _body(*args):
        operands = list(args)
        if partition_name is not None:
            operands.append(b2j.partition_id_tensor())
        return tuple(b2j._bass_exec_p.bind(
            *operands,
            out_avals=tuple(out_avals),
            in_names=tuple(all_names),
            out_names=tuple(out_names),
            lowering_input_output_aliases=(),
            sim_require_finite=True,
            sim_require_nnan=True,
            nc=nc,
        ))

    devices = jax.devices()[:8]
    mesh = Mesh(np.asarray(devices), ("core",))
    n_outs = len(out_names)
    fn = jax.jit(
        shard_map(
            _body, mesh=mesh,
            in_specs=(PartitionSpec("core"),) * (n_params + n_outs),
            out_specs=(PartitionSpec("core"),) * n_outs,
            check_rep=False,
        ),
        keep_unused=True,
    )
    return fn, in_names, out_avals, mesh


def bench(ins, iters=6):
    """Wall-clock timing of the jitted 8-core kernel (axon PJRT
    round-trip dominates; see trace path for true device time)."""
    import time
    import jax
    from jax.sharding import PartitionSpec, NamedSharding

    input_ = np.ascontiguousarray(ins["input"], dtype=np.float32)
    weight = np.ascontiguousarray(ins["weight"], dtype=np.float32)
    b2_full = np.ascontiguousarray(ins["bias"], dtype=np.float32) * np.float32(2.0)

    shard_arrays = {
        "x": np.concatenate([input_[(c // NCOL) * SM_FULL:(c // NCOL + 1) * SM_FULL, :] for c in range(8)], axis=0),
        "w": np.concatenate([weight[(c % NCOL) * SN_FULL:(c % NCOL + 1) * SN_FULL, :] for c in range(8)], axis=0),
        "b2": np.concatenate([b2_full[(c % NCOL) * SN_FULL:(c % NCOL + 1) * SN_FULL] for c in range(8)], axis=0),
    }

    nc = _get_nc()
    fn, in_names, out_avals, mesh = _make_runner(nc)
    sharding = NamedSharding(mesh, PartitionSpec("core"))
    dev_in = [jax.device_put(shard_arrays[nm], sharding) for nm in in_names]
    dev_zero = [
        jax.device_put(
            np.zeros((8 * a.shape[0], *a.shape[1:]), a.dtype), sharding)
        for a in out_avals
    ]
    out = fn(*dev_in, *dev_zero)
    jax.block_until_ready(out)
    best = float("inf")
    for _ in range(iters):
        t0 = time.perf_counter()
        out = fn(*dev_in, *dev_zero)
        jax.block_until_ready(out)
        best = min(best, time.perf_counter() - t0)
    print("bench[real wall]: %.3f ms" % (best * 1e3))
    return max(1, int(best * 1e9))


if __name__ == "__main__":
    import sys
    mode = sys.argv[1] if len(sys.argv) > 1 else "sim"
    if mode == "sim":
        # quick numerical validation in CoreSim on a small config
        from concourse.bass_interp import CoreSim
        SM, SN, K, CH = 256, 256, 512, 256
        nc = build_bass(SM, SN, K, CH)
        rng = np.random.default_rng(0)
        xin = rng.standard_normal((SM, K), dtype=np.float32)
        win = rng.uniform(-0.1, 0.1, (SN, K)).astype(np.float32)
        bin_ = rng.uniform(-0.1, 0.1, SN).astype(np.float32)

        sim = CoreSim(nc)
        sim.tensor("x")[:] = xin
        sim.tensor("w")[:] = win
        sim.tensor("b2")[:] = bin_ * 2.0
        sim.simulate(check_with_hw=False)
        got = np.array(sim.tensor("o"))

        def bfpq(v):
            g = v.reshape(v.shape[0], -1, GROUP).astype(np.float64)
            ma = np.abs(g).max(axis=-1, keepdims=True)
            e = np.floor(np.log2(np.where(ma > 0, ma, 1.0)))
            st = np.exp2(e - 6)
            qq = np.clip(np.round(g / st), -127, 127) * st
            return np.where(ma > 0, qq, 0.0).reshape(v.shape)

        exp = bfpq(xin) @ bfpq(win).T + 2.0 * bin_.astype(np.float64)
        err = np.abs(got.astype(np.float64) - exp)
        rel = err.max() / np.abs(exp).max()
        print("max abs err:", err.max(), "rel:", rel)
        assert rel < 1e-3, "numerical mismatch"
        print("SIM PASS")
    elif mode == "hw":
        import reference
        ins = {k: np.asarray(v) for k, v in reference.setup_inputs().items()}
        outp = kernel(**ins)
        print("out", outp.shape, outp.dtype)
